# revision 16
# baseline (speedup 1.0000x reference)
"""AdditiveAttention distributed Bass kernel for 8 TRN2 NeuronCores (v2, bf16).

Data-parallel over batch: B=8 samples -> 1 per core. Weights replicated.

Per-core math (S=2048, D=1024, H=16, HD=64):
  q = X @ W_qv + b_qv                 ; v = q
  k = X @ W_k + b_k
  alphas = softmax_h((q @ Wq_s + bq_s) * sc)       sc = 1/sqrt(HD)
  gq[d]  = sum_s alphas[s, h(d)] * q[s, d]         h(d) = d // 64
  p = k * gq                                        (broadcast over s)
  betas  = softmax_h((p @ Wk_s + bk_s) * sc)
  gk[d]  = gq[d] * sum_s betas[s, h(d)] * k[s, d]
  out = q + (q*gk) @ W_r + b_r

Key algebraic folds (avoid transposed-layout round trips):
  - logits_b = k @ (diag(gq) Wk_s) + bk_s          (p never materialized)
  - out      = q @ (I + diag(gk) W_r) + b_r        (residual folded into W_r)
  - gq_raw   = W_qv^T (X^T alphas) + b_qv colsum(alphas)   (q_nat never needed)

Layout: activations transposed (qT[d, s]) so big matmuls use natural weights
[c, d] as stationary and XT[c, s] as moving operand.  XT comes from the DMA
xbar transpose (bf16).  All matmuls bf16 (FWL weight loads) with f32 PSUM.
Host pre-casts X and weights to bf16.
"""

import math
import os
from contextlib import ExitStack

import numpy as np

B, S, D, H = 8, 2048, 1024, 16
HD = D // H
SCALE = 1.0 / math.sqrt(HD)
NCORES = 8
P = 128
NDB = D // P      # 8 d-blocks
NSB = S // P      # 16 s-blocks
NCC = D // P      # 8 contraction chunks
SH = 512          # psum free width for big matmuls
NSH = S // SH     # 4
NDH = D // SH     # 2

_CACHE = {}


def _build():
    import concourse.bacc as bacc
    import concourse.tile as tile
    import concourse.mybir as mybir

    f32 = mybir.dt.float32
    bf16 = mybir.dt.bfloat16
    AF = mybir.ActivationFunctionType
    ALU = mybir.AluOpType

    nc = bacc.Bacc("TRN2", target_bir_lowering=False, debug=False,
                   num_devices=NCORES)

    # bf16 inputs (host pre-cast)
    X = nc.dram_tensor("Xb", [S, D], bf16, kind="ExternalInput").ap()
    W_qv = nc.dram_tensor("W_qvb", [D, D], bf16, kind="ExternalInput").ap()
    W_k = nc.dram_tensor("W_kb", [D, D], bf16, kind="ExternalInput").ap()
    W_r = nc.dram_tensor("W_rb", [D, D], bf16, kind="ExternalInput").ap()
    Wq_s = nc.dram_tensor("Wq_sb", [D, H], bf16, kind="ExternalInput").ap()
    Wk_s = nc.dram_tensor("Wk_sb", [D, H], bf16, kind="ExternalInput").ap()
    bq_sb = nc.dram_tensor("bq_sbf", [H], bf16, kind="ExternalInput").ap()
    bk_sb = nc.dram_tensor("bk_sbf", [H], bf16, kind="ExternalInput").ap()
    br_b = nc.dram_tensor("b_rbf", [D], bf16, kind="ExternalInput").ap()
    bqv_b = nc.dram_tensor("b_qvbf", [D], bf16, kind="ExternalInput").ap()
    bkv_b = nc.dram_tensor("b_kbf", [D], bf16, kind="ExternalInput").ap()
    # f32 biases for per-partition epilogues
    b_qv = nc.dram_tensor("b_qv", [D], f32, kind="ExternalInput").ap()
    b_k = nc.dram_tensor("b_k", [D], f32, kind="ExternalInput").ap()
    OUT = nc.dram_tensor("out", [S, D], f32, kind="ExternalOutput").ap()

    with tile.TileContext(nc) as tc, ExitStack() as ctx:
        sbp = ctx.enter_context(tc.tile_pool(name="sbp", bufs=1))
        psp = ctx.enter_context(tc.tile_pool(name="psp", bufs=1, space="PSUM"))

        def st(shape, dt_, tag, bufs=1):
            return sbp.tile(shape, dt_, tag=tag, bufs=bufs, name=tag)

        def pt_(shape, tag, bufs):
            return psp.tile(shape, f32, tag=tag, bufs=bufs, name=tag)

        # ---------- constants / biases / small weights ----------
        ones_row = st([1, P], bf16, "ones_row")
        nc.gpsimd.memset(ones_row[:], 1.0)
        ones_col = st([P, 1], bf16, "ones_col")
        nc.gpsimd.memset(ones_col[:], 1.0)
        eye_bf = st([P, P], bf16, "eye_bf")
        nc.gpsimd.memset(eye_bf[:], 1.0)
        nc.gpsimd.affine_select(eye_bf[:], eye_bf[:], pattern=[[1, P]],
                                compare_op=ALU.is_equal, fill=0.0,
                                base=0, channel_multiplier=-1)


        # ---------- small persistent intermediates ----------
        aE = st([P, NSB * H], f32, "aE")
        Za = st([P, NSB], f32, "Za")
        rZa = st([P, NSB], f32, "rZa")
        alpha = st([P, NSB * H], bf16, "alpha")
        bEx = st([P, NSB * H], f32, "bEx")
        Zb = st([P, NSB], f32, "Zb")
        rZb = st([P, NSB], f32, "rZb")
        beta = st([P, NSB * H], bf16, "beta")
        Asb_q = st([P, NCC * H], bf16, "Asbq")
        Ssb_q = st([1, H], bf16, "Ssbq")
        Asb_k = st([P, NCC * H], bf16, "Asbk")
        Ssb_k = st([1, H], bf16, "Ssbk")
        gq = st([P, NDB], f32, "gq")
        gkd = st([P, NDB], f32, "gkd")
        gk = st([P, NDB], f32, "gk")

        # ---------- big persistent activations / resident data ----------
        xt = st([P, NCC * S], bf16, "xt")   # X^T, chunk cc at cols cc*S
        qt = st([P, NDB * S], bf16, "qt")   # q^T, d-block j at cols j*S
        kt = st([P, NDB * S], bf16, "kt")   # k^T
        xnat = st([P, NSB * D], bf16, "xnat")  # natural X, s-block si at si*D
        wqv_all = st([P, NCC * D], bf16, "wqv_all")
        wk_all = st([P, NCC * D], bf16, "wk_all")
        wr_all = st([P, NCC * D], bf16, "wr_all")

        # DMA issue order = SP program order: XT(sh0) -> W_qv -> xnat ->
        # XT(rest) -> W_k -> small weights/biases -> W_r.
        for cc in range(NCC):
            nc.sync.dma_start_transpose(
                xt[:, cc * S: cc * S + SH],
                X[0:SH, cc * P:(cc + 1) * P])
        nc.scalar.dma_start(
            wqv_all[:].rearrange("p (cc d) -> p cc d", cc=NCC),
            W_qv.rearrange("(cc p) d -> p cc d", p=P))
        nc.gpsimd.dma_start(
            xnat[:].rearrange("p (si c) -> p si c", si=NSB),
            X.rearrange("(si p) c -> p si c", p=P))
        for cc in range(NCC):
            nc.sync.dma_start_transpose(
                xt[:, cc * S + SH: (cc + 1) * S],
                X[SH:S, cc * P:(cc + 1) * P])
        nc.scalar.dma_start(
            wk_all[:].rearrange("p (cc d) -> p cc d", cc=NCC),
            W_k.rearrange("(cc p) d -> p cc d", p=P))

        bqv_pp = st([P, NDB], f32, "bqv_pp")
        nc.scalar.dma_start(bqv_pp[:], b_qv.rearrange("(j p) -> p j", p=P))
        bk_pp = st([P, NDB], f32, "bk_pp")
        nc.scalar.dma_start(bk_pp[:], b_k.rearrange("(j p) -> p j", p=P))
        bqv_row = st([1, D], bf16, "bqv_row")
        nc.scalar.dma_start(bqv_row[:], bqv_b.unsqueeze(0))
        bk_row = st([1, D], bf16, "bk_row")
        nc.scalar.dma_start(bk_row[:], bkv_b.unsqueeze(0))
        br_row = st([1, D], bf16, "br_row")
        nc.scalar.dma_start(br_row[:], br_b.unsqueeze(0))
        bqs_row = st([1, H], bf16, "bqs_row")
        nc.scalar.dma_start(bqs_row[:], bq_sb.unsqueeze(0))
        bks_row = st([1, H], bf16, "bks_row")
        nc.scalar.dma_start(bks_row[:], bk_sb.unsqueeze(0))
        wqs_sb = st([P, NDB * H], bf16, "wqs_sb")
        nc.scalar.dma_start(wqs_sb[:].rearrange("p (j h) -> p j h", j=NDB),
                          Wq_s.rearrange("(j p) h -> p j h", p=P))
        wks_sb = st([P, NDB * H], bf16, "wks_sb")   # becomes diag(gq)-scaled
        nc.scalar.dma_start(wks_sb[:].rearrange("p (j h) -> p j h", j=NDB),
                          Wk_s.rearrange("(j p) h -> p j h", p=P))

        nc.gpsimd.dma_start(
            wr_all[:].rearrange("p (cc d) -> p cc d", cc=NCC),
            W_r.rearrange("(cc p) d -> p cc d", p=P))

        # ---------- phases 2+3: qT / kT projections ----------
        def project(wall, bias_pp, dst):
            for j in range(NDB):
                for sh in range(NSH):
                    ps = pt_([P, SH], "big", 4)
                    for cc in range(NCC):
                        nc.tensor.matmul(
                            ps[:], wall[:, cc * D + j * P: cc * D + j * P + P],
                            xt[:, cc * S + sh * SH: cc * S + sh * SH + SH],
                            start=(cc == 0), stop=(cc == NCC - 1))
                    nc.vector.tensor_scalar(
                        dst[:, j * S + sh * SH: j * S + sh * SH + SH], ps[:],
                        bias_pp[:, j:j + 1], None, ALU.add)

        project(wqv_all, bqv_pp, qt)

        # ---------- logits + softmax (shared for alphas / betas) ----------
        def softmax_weights(src_t, w16, brow, eE, Z, rZ, wout, pe_filler=None):
            lg = pt_([P, NSB * H], "small", 3)
            for sb in range(NSB):
                for j in range(NDB):
                    nc.tensor.matmul(
                        lg[:, sb * H:(sb + 1) * H],
                        src_t[:, j * S + sb * P: j * S + sb * P + P],
                        w16[:, j * H:(j + 1) * H],
                        start=(sb == 0 and j == 0), stop=False)
                nc.tensor.matmul(
                    lg[:, sb * H:(sb + 1) * H],
                    ones_row[:1, :], brow[:1, :],
                    start=False, stop=(sb == NSB - 1))
            if pe_filler is not None:
                pe_filler()
            nc.scalar.activation(eE[:], lg[:], AF.Exp, bias=0.0, scale=SCALE)
            nc.vector.reduce_sum(
                Z[:].unsqueeze(2),
                eE[:].rearrange("p (sb h) -> p sb h", sb=NSB),
                axis=mybir.AxisListType.X)
            nc.vector.reciprocal(rZ[:], Z[:])
            nc.vector.tensor_tensor(
                wout[:].rearrange("p (sb h) -> p sb h", sb=NSB),
                eE[:].rearrange("p (sb h) -> p sb h", sb=NSB),
                rZ[:].unsqueeze(2).broadcast_to([P, NSB, H]),
                ALU.mult)

        # ---------- gq_raw = W^T (X^T w) + b colsum(w), extract diagonal ----------
        def weighted_sum(weights_sb, wall, b_row, Asb, Ssb, g):
            Aps = pt_([P, NCC * H], "small", 3)
            Sps = pt_([1, H], "small", 3)
            for si in range(NSB):
                for cb in range(NCC):
                    nc.tensor.matmul(
                        Aps[:, cb * H:(cb + 1) * H],
                        xnat[:, si * D + cb * P: si * D + cb * P + P],
                        weights_sb[:, si * H:(si + 1) * H],
                        start=(si == 0 and cb == 0),
                        stop=(si == NSB - 1 and cb == NCC - 1))
                nc.tensor.matmul(
                    Sps[:1, :], ones_col[:, :1],
                    weights_sb[:, si * H:(si + 1) * H],
                    start=(si == 0), stop=(si == NSB - 1))
            nc.vector.tensor_copy(Asb[:], Aps[:])
            nc.vector.tensor_copy(Ssb[:1, :], Sps[:1, :])
            graw = pt_([P, NDB * H], "small", 3)
            for j in range(NDB):
                for cc in range(NCC):
                    nc.tensor.matmul(
                        graw[:, j * H:(j + 1) * H],
                        wall[:, cc * D + j * P: cc * D + j * P + P],
                        Asb[:, cc * H:(cc + 1) * H],
                        start=(j == 0 and cc == 0), stop=False)
                nc.tensor.matmul(
                    graw[:, j * H:(j + 1) * H],
                    b_row[:1, j * P:(j + 1) * P], Ssb[:1, :],
                    start=False, stop=(j == NDB - 1))
            for j in range(NDB):
                c0 = j * H + 2 * j
                nc.vector.tensor_copy(g[0:64, j:j + 1], graw[0:64, c0:c0 + 1])
                nc.vector.tensor_copy(g[64:P, j:j + 1], graw[64:P, c0 + 1:c0 + 2])

        # alphas (k-projection emitted between logits and exp so the PE has
        # dense work while ACT/DVE run the softmax tail)
        softmax_weights(qt, wqs_sb, bqs_row, aE, Za, rZa, alpha,
                        pe_filler=lambda: project(wk_all, bk_pp, kt))
        # gq
        weighted_sum(alpha, wqv_all, bqv_row, Asb_q, Ssb_q, gq)
        # fold gq into Wk_s  ->  logits_b from kT directly
        for j in range(NDB):
            nc.vector.tensor_scalar(
                wks_sb[:, j * H:(j + 1) * H], wks_sb[:, j * H:(j + 1) * H],
                gq[:, j:j + 1], None, ALU.mult)
        # betas  (logits_b = k @ (diag(gq) Wk_s) + bk_s)
        softmax_weights(kt, wks_sb, bks_row, bEx, Zb, rZb, beta)
        # gk = gq * (W_k^T (X^T beta) + b_k colsum(beta)) diag
        weighted_sum(beta, wk_all, bk_row, Asb_k, Ssb_k, gkd)
        nc.vector.tensor_mul(gk[:], gq[:], gkd[:])

        # ---------- fold residual + gk into W_r:  W_r' = diag(gk) W_r + I ----------
        for cc in range(NCC):
            nc.vector.tensor_scalar(
                wr_all[:, cc * D:(cc + 1) * D], wr_all[:, cc * D:(cc + 1) * D],
                gk[:, cc:cc + 1], None, ALU.mult)
            nc.vector.tensor_add(
                wr_all[:, cc * D + cc * P: cc * D + (cc + 1) * P],
                wr_all[:, cc * D + cc * P: cc * D + (cc + 1) * P], eye_bf[:])

        # ---------- final: out = q @ W_r' + b_r ----------
        for sb in range(NSB):
            for dh in range(NDH):
                ps = pt_([P, SH], "big", 4)
                nc.tensor.matmul(
                    ps[:], ones_row[:1, :],
                    br_row[:1, dh * SH:(dh + 1) * SH],
                    start=True, stop=False)
                for cc in range(NCC):
                    nc.tensor.matmul(
                        ps[:], qt[:, cc * S + sb * P: cc * S + sb * P + P],
                        wr_all[:, cc * D + dh * SH: cc * D + dh * SH + SH],
                        start=False, stop=(cc == NCC - 1))
                ob = st([P, SH], f32, "ob", bufs=3)
                nc.scalar.copy(ob[:], ps[:])
                nc.sync.dma_start(
                    OUT[sb * P:(sb + 1) * P, dh * SH:(dh + 1) * SH], ob[:])

    nc.compile()
    return nc


def _get_nc():
    if "nc" not in _CACHE:
        _CACHE["nc"] = _build()
    return _CACHE["nc"]


def _prep_inputs(inputs):
    import ml_dtypes
    bf = ml_dtypes.bfloat16

    def f(k):
        return np.ascontiguousarray(np.asarray(inputs[k], dtype=np.float32))

    def c(a):
        return np.ascontiguousarray(np.asarray(a, dtype=np.float32).astype(bf))

    common = {
        "W_qvb": c(inputs["W_qv"]), "W_kb": c(inputs["W_k"]),
        "W_rb": c(inputs["W_r"]), "Wq_sb": c(inputs["Wq_s"]),
        "Wk_sb": c(inputs["Wk_s"]), "bq_sbf": c(inputs["bq_s"]),
        "bk_sbf": c(inputs["bk_s"]), "b_rbf": c(inputs["b_r"]),
        "b_qvbf": c(inputs["b_qv"]), "b_kbf": c(inputs["b_k"]),
        "b_qv": f("b_qv"), "b_k": f("b_k"),
    }
    in_maps = []
    for b in range(NCORES):
        m = dict(common)
        m["Xb"] = c(inputs["X"][b])
        in_maps.append(m)
    return in_maps


def run(inputs, trace=False):
    from concourse.bass_utils import run_bass_kernel_spmd

    nc = _get_nc()
    in_maps = _prep_inputs(inputs)
    res = run_bass_kernel_spmd(nc, in_maps, core_ids=list(range(NCORES)),
                               trace=trace)
    _CACHE["last_results"] = res
    out = np.stack([res.results[b]["out"] for b in range(NCORES)], axis=0)
    return out


def kernel(**inputs):
    trace = os.environ.get("KTRACE", "0") == "1"
    return run(inputs, trace=trace)


# revision 17
# speedup vs baseline: 1.2485x; 1.2485x over previous
"""AdditiveAttention distributed Bass kernel for 8 TRN2 NeuronCores (v2, bf16).

Data-parallel over batch: B=8 samples -> 1 per core. Weights replicated.

Per-core math (S=2048, D=1024, H=16, HD=64):
  q = X @ W_qv + b_qv                 ; v = q
  k = X @ W_k + b_k
  alphas = softmax_h((q @ Wq_s + bq_s) * sc)       sc = 1/sqrt(HD)
  gq[d]  = sum_s alphas[s, h(d)] * q[s, d]         h(d) = d // 64
  p = k * gq                                        (broadcast over s)
  betas  = softmax_h((p @ Wk_s + bk_s) * sc)
  gk[d]  = gq[d] * sum_s betas[s, h(d)] * k[s, d]
  out = q + (q*gk) @ W_r + b_r

Key algebraic folds (avoid transposed-layout round trips):
  - logits_b = k @ (diag(gq) Wk_s) + bk_s          (p never materialized)
  - out      = q @ (I + diag(gk) W_r) + b_r        (residual folded into W_r)
  - gq_raw   = W_qv^T (X^T alphas) + b_qv colsum(alphas)   (q_nat never needed)

Layout: activations transposed (qT[d, s]) so big matmuls use natural weights
[c, d] as stationary and XT[c, s] as moving operand.  XT comes from the DMA
xbar transpose (bf16).  All matmuls bf16 (FWL weight loads) with f32 PSUM.
Host pre-casts X and weights to bf16.
"""

import math
import os
from contextlib import ExitStack

import numpy as np

B, S, D, H = 8, 2048, 1024, 16
HD = D // H
SCALE = 1.0 / math.sqrt(HD)
NCORES = 8
P = 128
NDB = D // P      # 8 d-blocks
NSB = S // P      # 16 s-blocks
NCC = D // P      # 8 contraction chunks
SH = 512          # psum free width for big matmuls
NSH = S // SH     # 4
NDH = D // SH     # 2

_CACHE = {}


def _build():
    import concourse.bacc as bacc
    import concourse.tile as tile
    import concourse.mybir as mybir

    f32 = mybir.dt.float32
    bf16 = mybir.dt.bfloat16
    AF = mybir.ActivationFunctionType
    ALU = mybir.AluOpType

    nc = bacc.Bacc("TRN2", target_bir_lowering=False, debug=False,
                   num_devices=NCORES)

    # bf16 inputs (host pre-cast)
    X = nc.dram_tensor("Xb", [S, D], bf16, kind="ExternalInput").ap()
    XT = nc.dram_tensor("XTb", [D, S], bf16, kind="ExternalInput").ap()
    W_qv = nc.dram_tensor("W_qvb", [D, D], bf16, kind="ExternalInput").ap()
    W_k = nc.dram_tensor("W_kb", [D, D], bf16, kind="ExternalInput").ap()
    W_r = nc.dram_tensor("W_rb", [D, D], bf16, kind="ExternalInput").ap()
    Wq_s = nc.dram_tensor("Wq_sb", [D, H], bf16, kind="ExternalInput").ap()
    Wk_s = nc.dram_tensor("Wk_sb", [D, H], bf16, kind="ExternalInput").ap()
    bq_sb = nc.dram_tensor("bq_sbf", [H], bf16, kind="ExternalInput").ap()
    bk_sb = nc.dram_tensor("bk_sbf", [H], bf16, kind="ExternalInput").ap()
    br_b = nc.dram_tensor("b_rbf", [D], bf16, kind="ExternalInput").ap()
    bqv_b = nc.dram_tensor("b_qvbf", [D], bf16, kind="ExternalInput").ap()
    bkv_b = nc.dram_tensor("b_kbf", [D], bf16, kind="ExternalInput").ap()
    # f32 biases for per-partition epilogues
    b_qv = nc.dram_tensor("b_qv", [D], f32, kind="ExternalInput").ap()
    b_k = nc.dram_tensor("b_k", [D], f32, kind="ExternalInput").ap()
    OUT = nc.dram_tensor("out", [S, D], f32, kind="ExternalOutput").ap()

    with tile.TileContext(nc) as tc, ExitStack() as ctx:
        sbp = ctx.enter_context(tc.tile_pool(name="sbp", bufs=1))
        psp = ctx.enter_context(tc.tile_pool(name="psp", bufs=1, space="PSUM"))

        def st(shape, dt_, tag, bufs=1):
            return sbp.tile(shape, dt_, tag=tag, bufs=bufs, name=tag)

        def pt_(shape, tag, bufs):
            return psp.tile(shape, f32, tag=tag, bufs=bufs, name=tag)

        # ---------- constants / biases / small weights ----------
        ones_row = st([1, P], bf16, "ones_row")
        nc.gpsimd.memset(ones_row[:], 1.0)
        ones_col = st([P, 1], bf16, "ones_col")
        nc.gpsimd.memset(ones_col[:], 1.0)
        eye_bf = st([P, P], bf16, "eye_bf")
        nc.gpsimd.memset(eye_bf[:], 1.0)
        nc.gpsimd.affine_select(eye_bf[:], eye_bf[:], pattern=[[1, P]],
                                compare_op=ALU.is_equal, fill=0.0,
                                base=0, channel_multiplier=-1)


        # ---------- small persistent intermediates ----------
        aE = st([P, NSB * H], f32, "aE")
        Za = st([P, NSB], f32, "Za")
        rZa = st([P, NSB], f32, "rZa")
        alpha = st([P, NSB * H], bf16, "alpha")
        bEx = st([P, NSB * H], f32, "bEx")
        Zb = st([P, NSB], f32, "Zb")
        rZb = st([P, NSB], f32, "rZb")
        beta = st([P, NSB * H], bf16, "beta")
        Asb_q = st([P, NCC * H], bf16, "Asbq")
        Ssb_q = st([1, H], bf16, "Ssbq")
        Asb_k = st([P, NCC * H], bf16, "Asbk")
        Ssb_k = st([1, H], bf16, "Ssbk")
        gq = st([P, NDB], f32, "gq")
        gkd = st([P, NDB], f32, "gkd")
        gk = st([P, NDB], f32, "gk")

        # ---------- big persistent activations / resident data ----------
        xt = st([P, NCC * S], bf16, "xt")   # X^T, chunk cc at cols cc*S
        qt = st([P, NDB * S], bf16, "qt")   # q^T, d-block j at cols j*S
        kt = st([P, NDB * S], bf16, "kt")   # k^T
        xnat = st([P, NSB * D], bf16, "xnat")  # natural X, s-block si at si*D
        wqv_all = st([P, NCC * D], bf16, "wqv_all")
        wk_all = st([P, NCC * D], bf16, "wk_all")
        wr_all = st([P, NCC * D], bf16, "wr_all")

        # Queue split: gpsimd SWDGE carries the startup-critical XT + W_qv,
        # SP carries xnat + W_k (+ output stores later), ACT carries the
        # late-needed W_r and small tensors.
        nc.gpsimd.dma_start(
            xt[:].rearrange("p (cc s) -> p cc s", cc=NCC),
            XT.rearrange("(cc p) s -> p cc s", p=P))
        nc.gpsimd.dma_start(
            wqv_all[:].rearrange("p (cc d) -> p cc d", cc=NCC),
            W_qv.rearrange("(cc p) d -> p cc d", p=P))
        nc.sync.dma_start(
            xnat[:].rearrange("p (si c) -> p si c", si=NSB),
            X.rearrange("(si p) c -> p si c", p=P))
        nc.sync.dma_start(
            wk_all[:].rearrange("p (cc d) -> p cc d", cc=NCC),
            W_k.rearrange("(cc p) d -> p cc d", p=P))

        bqv_pp = st([P, NDB], f32, "bqv_pp")
        nc.scalar.dma_start(bqv_pp[:], b_qv.rearrange("(j p) -> p j", p=P))
        bk_pp = st([P, NDB], f32, "bk_pp")
        nc.scalar.dma_start(bk_pp[:], b_k.rearrange("(j p) -> p j", p=P))
        bqv_row = st([1, D], bf16, "bqv_row")
        nc.scalar.dma_start(bqv_row[:], bqv_b.unsqueeze(0))
        bk_row = st([1, D], bf16, "bk_row")
        nc.scalar.dma_start(bk_row[:], bkv_b.unsqueeze(0))
        br_row = st([1, D], bf16, "br_row")
        nc.scalar.dma_start(br_row[:], br_b.unsqueeze(0))
        bqs_row = st([1, H], bf16, "bqs_row")
        nc.scalar.dma_start(bqs_row[:], bq_sb.unsqueeze(0))
        bks_row = st([1, H], bf16, "bks_row")
        nc.scalar.dma_start(bks_row[:], bk_sb.unsqueeze(0))
        wqs_sb = st([P, NDB * H], bf16, "wqs_sb")
        nc.scalar.dma_start(wqs_sb[:].rearrange("p (j h) -> p j h", j=NDB),
                          Wq_s.rearrange("(j p) h -> p j h", p=P))
        wks_sb = st([P, NDB * H], bf16, "wks_sb")   # becomes diag(gq)-scaled
        nc.scalar.dma_start(wks_sb[:].rearrange("p (j h) -> p j h", j=NDB),
                          Wk_s.rearrange("(j p) h -> p j h", p=P))

        nc.scalar.dma_start(
            wr_all[:].rearrange("p (cc d) -> p cc d", cc=NCC),
            W_r.rearrange("(cc p) d -> p cc d", p=P))

        # ---------- phases 2+3: qT / kT projections ----------
        def project(wall, bias_pp, dst):
            for j in range(NDB):
                for sh in range(NSH):
                    ps = pt_([P, SH], "big", 4)
                    for cc in range(NCC):
                        nc.tensor.matmul(
                            ps[:], wall[:, cc * D + j * P: cc * D + j * P + P],
                            xt[:, cc * S + sh * SH: cc * S + sh * SH + SH],
                            start=(cc == 0), stop=(cc == NCC - 1))
                    nc.vector.tensor_scalar(
                        dst[:, j * S + sh * SH: j * S + sh * SH + SH], ps[:],
                        bias_pp[:, j:j + 1], None, ALU.add)

        project(wqv_all, bqv_pp, qt)

        # ---------- logits + softmax (shared for alphas / betas) ----------
        def softmax_weights(src_t, w16, brow, eE, Z, rZ, wout, pe_filler=None):
            lg = pt_([P, NSB * H], "small", 3)
            for sb in range(NSB):
                for j in range(NDB):
                    nc.tensor.matmul(
                        lg[:, sb * H:(sb + 1) * H],
                        src_t[:, j * S + sb * P: j * S + sb * P + P],
                        w16[:, j * H:(j + 1) * H],
                        start=(sb == 0 and j == 0), stop=False)
                nc.tensor.matmul(
                    lg[:, sb * H:(sb + 1) * H],
                    ones_row[:1, :], brow[:1, :],
                    start=False, stop=(sb == NSB - 1))
            if pe_filler is not None:
                pe_filler()
            nc.scalar.activation(eE[:], lg[:], AF.Exp, bias=0.0, scale=SCALE)
            nc.vector.reduce_sum(
                Z[:].unsqueeze(2),
                eE[:].rearrange("p (sb h) -> p sb h", sb=NSB),
                axis=mybir.AxisListType.X)
            nc.vector.reciprocal(rZ[:], Z[:])
            nc.vector.tensor_tensor(
                wout[:].rearrange("p (sb h) -> p sb h", sb=NSB),
                eE[:].rearrange("p (sb h) -> p sb h", sb=NSB),
                rZ[:].unsqueeze(2).broadcast_to([P, NSB, H]),
                ALU.mult)

        # ---------- gq_raw = W^T (X^T w) + b colsum(w), extract diagonal ----------
        def weighted_sum(weights_sb, wall, b_row, Asb, Ssb, g):
            Aps = pt_([P, NCC * H], "small", 3)
            Sps = pt_([1, H], "small", 3)
            for si in range(NSB):
                for cb in range(NCC):
                    nc.tensor.matmul(
                        Aps[:, cb * H:(cb + 1) * H],
                        xnat[:, si * D + cb * P: si * D + cb * P + P],
                        weights_sb[:, si * H:(si + 1) * H],
                        start=(si == 0 and cb == 0),
                        stop=(si == NSB - 1 and cb == NCC - 1))
                nc.tensor.matmul(
                    Sps[:1, :], ones_col[:, :1],
                    weights_sb[:, si * H:(si + 1) * H],
                    start=(si == 0), stop=(si == NSB - 1))
            nc.vector.tensor_copy(Asb[:], Aps[:])
            nc.vector.tensor_copy(Ssb[:1, :], Sps[:1, :])
            graw = pt_([P, NDB * H], "small", 3)
            for j in range(NDB):
                for cc in range(NCC):
                    nc.tensor.matmul(
                        graw[:, j * H:(j + 1) * H],
                        wall[:, cc * D + j * P: cc * D + j * P + P],
                        Asb[:, cc * H:(cc + 1) * H],
                        start=(j == 0 and cc == 0), stop=False)
                nc.tensor.matmul(
                    graw[:, j * H:(j + 1) * H],
                    b_row[:1, j * P:(j + 1) * P], Ssb[:1, :],
                    start=False, stop=(j == NDB - 1))
            for j in range(NDB):
                c0 = j * H + 2 * j
                nc.vector.tensor_copy(g[0:64, j:j + 1], graw[0:64, c0:c0 + 1])
                nc.vector.tensor_copy(g[64:P, j:j + 1], graw[64:P, c0 + 1:c0 + 2])

        # alphas (k-projection emitted between logits and exp so the PE has
        # dense work while ACT/DVE run the softmax tail)
        softmax_weights(qt, wqs_sb, bqs_row, aE, Za, rZa, alpha,
                        pe_filler=lambda: project(wk_all, bk_pp, kt))
        # gq
        weighted_sum(alpha, wqv_all, bqv_row, Asb_q, Ssb_q, gq)
        # fold gq into Wk_s  ->  logits_b from kT directly
        for j in range(NDB):
            nc.vector.tensor_scalar(
                wks_sb[:, j * H:(j + 1) * H], wks_sb[:, j * H:(j + 1) * H],
                gq[:, j:j + 1], None, ALU.mult)
        # betas  (logits_b = k @ (diag(gq) Wk_s) + bk_s)
        softmax_weights(kt, wks_sb, bks_row, bEx, Zb, rZb, beta)
        # gk = gq * (W_k^T (X^T beta) + b_k colsum(beta)) diag
        weighted_sum(beta, wk_all, bk_row, Asb_k, Ssb_k, gkd)
        nc.vector.tensor_mul(gk[:], gq[:], gkd[:])

        # ---------- fold residual + gk into W_r:  W_r' = diag(gk) W_r + I ----------
        for cc in range(NCC):
            nc.vector.tensor_scalar(
                wr_all[:, cc * D:(cc + 1) * D], wr_all[:, cc * D:(cc + 1) * D],
                gk[:, cc:cc + 1], None, ALU.mult)
            nc.vector.tensor_add(
                wr_all[:, cc * D + cc * P: cc * D + (cc + 1) * P],
                wr_all[:, cc * D + cc * P: cc * D + (cc + 1) * P], eye_bf[:])

        # ---------- final: out = q @ W_r' + b_r ----------
        for sb in range(NSB):
            for dh in range(NDH):
                ps = pt_([P, SH], "big", 4)
                nc.tensor.matmul(
                    ps[:], ones_row[:1, :],
                    br_row[:1, dh * SH:(dh + 1) * SH],
                    start=True, stop=False)
                for cc in range(NCC):
                    nc.tensor.matmul(
                        ps[:], qt[:, cc * S + sb * P: cc * S + sb * P + P],
                        wr_all[:, cc * D + dh * SH: cc * D + dh * SH + SH],
                        start=False, stop=(cc == NCC - 1))
                ob = st([P, SH], f32, "ob", bufs=3)
                nc.scalar.copy(ob[:], ps[:])
                nc.sync.dma_start(
                    OUT[sb * P:(sb + 1) * P, dh * SH:(dh + 1) * SH], ob[:])

    nc.compile()
    return nc


def _get_nc():
    if "nc" not in _CACHE:
        _CACHE["nc"] = _build()
    return _CACHE["nc"]


def _prep_inputs(inputs):
    import ml_dtypes
    bf = ml_dtypes.bfloat16

    def f(k):
        return np.ascontiguousarray(np.asarray(inputs[k], dtype=np.float32))

    def c(a):
        return np.ascontiguousarray(np.asarray(a, dtype=np.float32).astype(bf))

    common = {
        "W_qvb": c(inputs["W_qv"]), "W_kb": c(inputs["W_k"]),
        "W_rb": c(inputs["W_r"]), "Wq_sb": c(inputs["Wq_s"]),
        "Wk_sb": c(inputs["Wk_s"]), "bq_sbf": c(inputs["bq_s"]),
        "bk_sbf": c(inputs["bk_s"]), "b_rbf": c(inputs["b_r"]),
        "b_qvbf": c(inputs["b_qv"]), "b_kbf": c(inputs["b_k"]),
        "b_qv": f("b_qv"), "b_k": f("b_k"),
    }
    in_maps = []
    for b in range(NCORES):
        m = dict(common)
        xb = np.asarray(inputs["X"][b], dtype=np.float32)
        m["Xb"] = c(xb)
        m["XTb"] = c(xb.T)
        in_maps.append(m)
    return in_maps


def run(inputs, trace=False):
    from concourse.bass_utils import run_bass_kernel_spmd

    nc = _get_nc()
    in_maps = _prep_inputs(inputs)
    res = run_bass_kernel_spmd(nc, in_maps, core_ids=list(range(NCORES)),
                               trace=trace)
    _CACHE["last_results"] = res
    out = np.stack([res.results[b]["out"] for b in range(NCORES)], axis=0)
    return out


def kernel(**inputs):
    trace = os.environ.get("KTRACE", "0") == "1"
    return run(inputs, trace=trace)


# revision 22
# speedup vs baseline: 1.2826x; 1.0273x over previous
"""AdditiveAttention distributed Bass kernel for 8 TRN2 NeuronCores (v2, bf16).

Data-parallel over batch: B=8 samples -> 1 per core. Weights replicated.

Per-core math (S=2048, D=1024, H=16, HD=64):
  q = X @ W_qv + b_qv                 ; v = q
  k = X @ W_k + b_k
  alphas = softmax_h((q @ Wq_s + bq_s) * sc)       sc = 1/sqrt(HD)
  gq[d]  = sum_s alphas[s, h(d)] * q[s, d]         h(d) = d // 64
  p = k * gq                                        (broadcast over s)
  betas  = softmax_h((p @ Wk_s + bk_s) * sc)
  gk[d]  = gq[d] * sum_s betas[s, h(d)] * k[s, d]
  out = q + (q*gk) @ W_r + b_r

Key algebraic folds (avoid transposed-layout round trips):
  - logits_b = k @ (diag(gq) Wk_s) + bk_s          (p never materialized)
  - out      = q @ (I + diag(gk) W_r) + b_r        (residual folded into W_r)
  - gq_raw   = W_qv^T (X^T alphas) + b_qv colsum(alphas)   (q_nat never needed)

Layout: activations transposed (qT[d, s]) so big matmuls use natural weights
[c, d] as stationary and XT[c, s] as moving operand.  XT comes from the DMA
xbar transpose (bf16).  All matmuls bf16 (FWL weight loads) with f32 PSUM.
Host pre-casts X and weights to bf16.
"""

import math
import os
from contextlib import ExitStack

import numpy as np

B, S, D, H = 8, 2048, 1024, 16
HD = D // H
SCALE = 1.0 / math.sqrt(HD)
NCORES = 8
P = 128
NDB = D // P      # 8 d-blocks
NSB = S // P      # 16 s-blocks
NCC = D // P      # 8 contraction chunks
SH = 512          # psum free width for big matmuls
NSH = S // SH     # 4
NDH = D // SH     # 2

_CACHE = {}


def _build():
    import concourse.bacc as bacc
    import concourse.tile as tile
    import concourse.mybir as mybir

    f32 = mybir.dt.float32
    bf16 = mybir.dt.bfloat16
    AF = mybir.ActivationFunctionType
    ALU = mybir.AluOpType

    nc = bacc.Bacc("TRN2", target_bir_lowering=False, debug=False,
                   num_devices=NCORES)

    # bf16 inputs (host pre-cast)
    X = nc.dram_tensor("Xb", [S, D], bf16, kind="ExternalInput").ap()
    XT = nc.dram_tensor("XTb", [D, S], bf16, kind="ExternalInput").ap()
    W_qv = nc.dram_tensor("W_qvb", [D, D], bf16, kind="ExternalInput").ap()
    W_k = nc.dram_tensor("W_kb", [D, D], bf16, kind="ExternalInput").ap()
    W_r = nc.dram_tensor("W_rb", [D, D], bf16, kind="ExternalInput").ap()
    Wq_s = nc.dram_tensor("Wq_sb", [D, H], bf16, kind="ExternalInput").ap()
    Wk_s = nc.dram_tensor("Wk_sb", [D, H], bf16, kind="ExternalInput").ap()
    bq_sb = nc.dram_tensor("bq_sbf", [H], bf16, kind="ExternalInput").ap()
    bk_sb = nc.dram_tensor("bk_sbf", [H], bf16, kind="ExternalInput").ap()
    br_b = nc.dram_tensor("b_rbf", [D], bf16, kind="ExternalInput").ap()
    bqv_b = nc.dram_tensor("b_qvbf", [D], bf16, kind="ExternalInput").ap()
    bkv_b = nc.dram_tensor("b_kbf", [D], bf16, kind="ExternalInput").ap()
    # f32 biases for per-partition epilogues
    b_qv = nc.dram_tensor("b_qv", [D], f32, kind="ExternalInput").ap()
    b_k = nc.dram_tensor("b_k", [D], f32, kind="ExternalInput").ap()
    OUT = nc.dram_tensor("out", [S, D], f32, kind="ExternalOutput").ap()

    with tile.TileContext(nc) as tc, ExitStack() as ctx:
        sbp = ctx.enter_context(tc.tile_pool(name="sbp", bufs=1))
        psp = ctx.enter_context(tc.tile_pool(name="psp", bufs=1, space="PSUM"))

        def st(shape, dt_, tag, bufs=1):
            return sbp.tile(shape, dt_, tag=tag, bufs=bufs, name=tag)

        def pt_(shape, tag, bufs):
            return psp.tile(shape, f32, tag=tag, bufs=bufs, name=tag)


        # ---------- small persistent intermediates ----------
        aE = st([P, NSB * H], f32, "aE")
        Za = st([P, NSB], f32, "Za")
        rZa = st([P, NSB], f32, "rZa")
        alpha = st([P, NSB * H], bf16, "alpha")
        bEx = st([P, NSB * H], f32, "bEx")
        Zb = st([P, NSB], f32, "Zb")
        rZb = st([P, NSB], f32, "rZb")
        beta = st([P, NSB * H], bf16, "beta")
        Asb_q = st([P, NCC * H], bf16, "Asbq")
        Ssb_q = st([1, H], bf16, "Ssbq")
        Asb_k = st([P, NCC * H], bf16, "Asbk")
        Ssb_k = st([1, H], bf16, "Ssbk")
        gq = st([P, NDB], f32, "gq")
        gkd = st([P, NDB], f32, "gkd")
        gk = st([P, NDB], f32, "gk")

        # ---------- big persistent activations / resident data ----------
        xt = st([P, NCC * S], bf16, "xt")   # X^T, chunk cc at cols cc*S
        qt = st([P, NDB * S], bf16, "qt")   # q^T, d-block j at cols j*S
        kt = st([P, NDB * S], bf16, "kt")   # k^T
        xnat = st([P, NSB * D], bf16, "xnat")  # natural X, s-block si at si*D
        wqv_all = st([P, NCC * D], bf16, "wqv_all")
        wk_all = st([P, NCC * D], bf16, "wk_all")
        wr_all = st([P, NCC * D], bf16, "wr_all")

        # Queue split: the startup-critical XT + W_qv are halved across the
        # SP HWDGE queue and the gpsimd SWDGE queue so both pump in parallel;
        # xnat + W_k follow on SP, W_r + small tensors on ACT.
        HB = NCC // 2
        nc.sync.dma_start(
            xt[:, :HB * S].rearrange("p (cc s) -> p cc s", cc=HB),
            XT[:HB * P, :].rearrange("(cc p) s -> p cc s", p=P))
        nc.gpsimd.dma_start(
            xt[:, HB * S:].rearrange("p (cc s) -> p cc s", cc=HB),
            XT[HB * P:, :].rearrange("(cc p) s -> p cc s", p=P))
        nc.sync.dma_start(
            wqv_all[:, :HB * D].rearrange("p (cc d) -> p cc d", cc=HB),
            W_qv[:HB * P, :].rearrange("(cc p) d -> p cc d", p=P))
        nc.gpsimd.dma_start(
            wqv_all[:, HB * D:].rearrange("p (cc d) -> p cc d", cc=HB),
            W_qv[HB * P:, :].rearrange("(cc p) d -> p cc d", p=P))
        nc.sync.dma_start(
            xnat[:].rearrange("p (si c) -> p si c", si=NSB),
            X.rearrange("(si p) c -> p si c", p=P))
        nc.sync.dma_start(
            wk_all[:].rearrange("p (cc d) -> p cc d", cc=NCC),
            W_k.rearrange("(cc p) d -> p cc d", p=P))

        # ---------- constants (gpsimd, after its DMAs are queued) ----------
        ones_row = st([1, P], bf16, "ones_row")
        nc.gpsimd.memset(ones_row[:], 1.0)
        ones_col = st([P, 1], bf16, "ones_col")
        nc.gpsimd.memset(ones_col[:], 1.0)
        ones16 = st([16, 1], bf16, "ones16")
        nc.gpsimd.memset(ones16[:], 1.0)
        eye_bf = st([P, P], bf16, "eye_bf")
        nc.gpsimd.memset(eye_bf[:], 1.0)
        nc.gpsimd.affine_select(eye_bf[:], eye_bf[:], pattern=[[1, P]],
                                compare_op=ALU.is_equal, fill=0.0,
                                base=0, channel_multiplier=-1)
        # head-selector mask: hmask[h, d] = 1 iff h == d//64   (16 partitions)
        # built as [d >= 64h] - [d >= 64(h+1)]  (walrus implements is_ge only)
        hmask = st([16, D], f32, "hmask")
        hm2 = st([16, D], f32, "hm2")
        nc.gpsimd.memset(hmask[:], 1.0)
        nc.gpsimd.memset(hm2[:], 1.0)
        nc.gpsimd.affine_select(hmask[:], hmask[:], pattern=[[1, D]],
                                compare_op=ALU.is_ge, fill=0.0,
                                base=0, channel_multiplier=-HD)
        nc.gpsimd.affine_select(hm2[:], hm2[:], pattern=[[1, D]],
                                compare_op=ALU.is_ge, fill=0.0,
                                base=-HD, channel_multiplier=-HD)
        nc.vector.tensor_sub(hmask[:], hmask[:], hm2[:])

        bqv_pp = st([P, NDB], f32, "bqv_pp")
        nc.scalar.dma_start(bqv_pp[:], b_qv.rearrange("(j p) -> p j", p=P))
        bk_pp = st([P, NDB], f32, "bk_pp")
        nc.scalar.dma_start(bk_pp[:], b_k.rearrange("(j p) -> p j", p=P))
        bqv_row = st([1, D], bf16, "bqv_row")
        nc.scalar.dma_start(bqv_row[:], bqv_b.unsqueeze(0))
        bk_row = st([1, D], bf16, "bk_row")
        nc.scalar.dma_start(bk_row[:], bkv_b.unsqueeze(0))
        br_row = st([1, D], bf16, "br_row")
        nc.scalar.dma_start(br_row[:], br_b.unsqueeze(0))
        bqs_row = st([1, H], bf16, "bqs_row")
        nc.scalar.dma_start(bqs_row[:], bq_sb.unsqueeze(0))
        bks_row = st([1, H], bf16, "bks_row")
        nc.scalar.dma_start(bks_row[:], bk_sb.unsqueeze(0))
        wqs_sb = st([P, NDB * H], bf16, "wqs_sb")
        nc.scalar.dma_start(wqs_sb[:].rearrange("p (j h) -> p j h", j=NDB),
                          Wq_s.rearrange("(j p) h -> p j h", p=P))
        wks_sb = st([P, NDB * H], bf16, "wks_sb")   # becomes diag(gq)-scaled
        nc.scalar.dma_start(wks_sb[:].rearrange("p (j h) -> p j h", j=NDB),
                          Wk_s.rearrange("(j p) h -> p j h", p=P))

        nc.scalar.dma_start(
            wr_all[:].rearrange("p (cc d) -> p cc d", cc=NCC),
            W_r.rearrange("(cc p) d -> p cc d", p=P))

        # ---------- phases 2+3: qT / kT projections ----------
        def project(wall, bias_pp, dst):
            for j in range(NDB):
                for sh in range(NSH):
                    ps = pt_([P, SH], "big", 3)
                    for cc in range(NCC):
                        nc.tensor.matmul(
                            ps[:], wall[:, cc * D + j * P: cc * D + j * P + P],
                            xt[:, cc * S + sh * SH: cc * S + sh * SH + SH],
                            start=(cc == 0), stop=(cc == NCC - 1))
                    nc.vector.tensor_scalar(
                        dst[:, j * S + sh * SH: j * S + sh * SH + SH], ps[:],
                        bias_pp[:, j:j + 1], None, ALU.add)

        project(wqv_all, bqv_pp, qt)

        # ---------- logits + softmax (shared for alphas / betas) ----------
        def softmax_weights(src_t, w16, brow, eE, Z, rZ, wout, pe_filler=None):
            lg = pt_([P, NSB * H], "small", 3)
            for sb in range(NSB):
                for j in range(NDB):
                    nc.tensor.matmul(
                        lg[:, sb * H:(sb + 1) * H],
                        src_t[:, j * S + sb * P: j * S + sb * P + P],
                        w16[:, j * H:(j + 1) * H],
                        start=(sb == 0 and j == 0), stop=False)
                nc.tensor.matmul(
                    lg[:, sb * H:(sb + 1) * H],
                    ones_row[:1, :], brow[:1, :],
                    start=False, stop=(sb == NSB - 1))
            if pe_filler is not None:
                pe_filler()
            nc.scalar.activation(eE[:], lg[:], AF.Exp, bias=0.0, scale=SCALE)
            nc.vector.reduce_sum(
                Z[:].unsqueeze(2),
                eE[:].rearrange("p (sb h) -> p sb h", sb=NSB),
                axis=mybir.AxisListType.X)
            nc.vector.reciprocal(rZ[:], Z[:])
            nc.vector.tensor_tensor(
                wout[:].rearrange("p (sb h) -> p sb h", sb=NSB),
                eE[:].rearrange("p (sb h) -> p sb h", sb=NSB),
                rZ[:].unsqueeze(2).broadcast_to([P, NSB, H]),
                ALU.mult)

        # ---------- gq_raw = W^T (X^T w) + b colsum(w), extract diagonal ----------
        def weighted_sum(weights_sb, wall, b_row, Asb, Ssb, g):
            Aps = pt_([P, NCC * H], "small", 3)
            Sps = pt_([1, H], "small", 3)
            for si in range(NSB):
                for cb in range(NCC):
                    nc.tensor.matmul(
                        Aps[:, cb * H:(cb + 1) * H],
                        xnat[:, si * D + cb * P: si * D + cb * P + P],
                        weights_sb[:, si * H:(si + 1) * H],
                        start=(si == 0 and cb == 0),
                        stop=(si == NSB - 1 and cb == NCC - 1))
                nc.tensor.matmul(
                    Sps[:1, :], ones_col[:, :1],
                    weights_sb[:, si * H:(si + 1) * H],
                    start=(si == 0), stop=(si == NSB - 1))
            nc.vector.tensor_copy(Asb[:], Aps[:])
            nc.vector.tensor_copy(Ssb[:1, :], Sps[:1, :])
            # gq_rawT[h, d] = (A^T W)[h, d] + S[h] b[d]   (16-col stationary)
            grawT = pt_([16, D], "grawT", 1)
            for dh in range(NDH):
                for cc in range(NCC):
                    nc.tensor.matmul(
                        grawT[:, dh * SH:(dh + 1) * SH],
                        Asb[:, cc * H:(cc + 1) * H],
                        wall[:, cc * D + dh * SH: cc * D + dh * SH + SH],
                        start=(cc == 0), stop=False)
                nc.tensor.matmul(
                    grawT[:, dh * SH:(dh + 1) * SH],
                    Ssb[:1, :], b_row[:1, dh * SH:(dh + 1) * SH],
                    start=False, stop=True)
            # diagonal extract: g[d] = gq_rawT[d//64, d]  via mask + ones-matmul
            msk = st([16, D], bf16, "msk", bufs=2)
            nc.vector.tensor_tensor(msk[:], grawT[:], hmask[:], ALU.mult)
            gps = pt_([P, NDB], "small", 3)
            for j in range(NDB):
                nc.tensor.matmul(
                    gps[:, j:j + 1],
                    msk[:, j * P:(j + 1) * P], ones16[:, :1],
                    start=(j == 0), stop=(j == NDB - 1))
            nc.vector.tensor_copy(g[:], gps[:])

        # alphas (k-projection emitted between logits and exp so the PE has
        # dense work while ACT/DVE run the softmax tail)
        softmax_weights(qt, wqs_sb, bqs_row, aE, Za, rZa, alpha,
                        pe_filler=lambda: project(wk_all, bk_pp, kt))
        # gq
        weighted_sum(alpha, wqv_all, bqv_row, Asb_q, Ssb_q, gq)
        # fold gq into Wk_s  ->  logits_b from kT directly
        for j in range(NDB):
            nc.vector.tensor_scalar(
                wks_sb[:, j * H:(j + 1) * H], wks_sb[:, j * H:(j + 1) * H],
                gq[:, j:j + 1], None, ALU.mult)
        # betas  (logits_b = k @ (diag(gq) Wk_s) + bk_s)
        softmax_weights(kt, wks_sb, bks_row, bEx, Zb, rZb, beta)
        # gk = gq * (W_k^T (X^T beta) + b_k colsum(beta)) diag
        weighted_sum(beta, wk_all, bk_row, Asb_k, Ssb_k, gkd)
        nc.vector.tensor_mul(gk[:], gq[:], gkd[:])

        # ---------- fold residual + gk into W_r:  W_r' = diag(gk) W_r + I ----------
        for cc in range(NCC):
            nc.vector.tensor_scalar(
                wr_all[:, cc * D:(cc + 1) * D], wr_all[:, cc * D:(cc + 1) * D],
                gk[:, cc:cc + 1], None, ALU.mult)
            nc.vector.tensor_add(
                wr_all[:, cc * D + cc * P: cc * D + (cc + 1) * P],
                wr_all[:, cc * D + cc * P: cc * D + (cc + 1) * P], eye_bf[:])

        # ---------- final: out = q @ W_r' + b_r ----------
        for sb in range(NSB):
            for dh in range(NDH):
                ps = pt_([P, SH], "big", 3)
                nc.tensor.matmul(
                    ps[:], ones_row[:1, :],
                    br_row[:1, dh * SH:(dh + 1) * SH],
                    start=True, stop=False)
                for cc in range(NCC):
                    nc.tensor.matmul(
                        ps[:], qt[:, cc * S + sb * P: cc * S + sb * P + P],
                        wr_all[:, cc * D + dh * SH: cc * D + dh * SH + SH],
                        start=False, stop=(cc == NCC - 1))
                ob = st([P, SH], f32, "ob", bufs=3)
                nc.scalar.copy(ob[:], ps[:])
                nc.sync.dma_start(
                    OUT[sb * P:(sb + 1) * P, dh * SH:(dh + 1) * SH], ob[:])

    nc.compile()
    return nc


def _get_nc():
    if "nc" not in _CACHE:
        _CACHE["nc"] = _build()
    return _CACHE["nc"]


def _prep_inputs(inputs):
    import ml_dtypes
    bf = ml_dtypes.bfloat16

    def f(k):
        return np.ascontiguousarray(np.asarray(inputs[k], dtype=np.float32))

    def c(a):
        return np.ascontiguousarray(np.asarray(a, dtype=np.float32).astype(bf))

    common = {
        "W_qvb": c(inputs["W_qv"]), "W_kb": c(inputs["W_k"]),
        "W_rb": c(inputs["W_r"]), "Wq_sb": c(inputs["Wq_s"]),
        "Wk_sb": c(inputs["Wk_s"]), "bq_sbf": c(inputs["bq_s"]),
        "bk_sbf": c(inputs["bk_s"]), "b_rbf": c(inputs["b_r"]),
        "b_qvbf": c(inputs["b_qv"]), "b_kbf": c(inputs["b_k"]),
        "b_qv": f("b_qv"), "b_k": f("b_k"),
    }
    in_maps = []
    for b in range(NCORES):
        m = dict(common)
        xb = np.asarray(inputs["X"][b], dtype=np.float32)
        m["Xb"] = c(xb)
        m["XTb"] = c(xb.T)
        in_maps.append(m)
    return in_maps


def run(inputs, trace=False):
    from concourse.bass_utils import run_bass_kernel_spmd

    nc = _get_nc()
    in_maps = _prep_inputs(inputs)
    res = run_bass_kernel_spmd(nc, in_maps, core_ids=list(range(NCORES)),
                               trace=trace)
    _CACHE["last_results"] = res
    out = np.stack([res.results[b]["out"] for b in range(NCORES)], axis=0)
    return out


def kernel(**inputs):
    trace = os.environ.get("KTRACE", "0") == "1"
    return run(inputs, trace=trace)


# revision 24
# speedup vs baseline: 1.2975x; 1.0116x over previous
"""AdditiveAttention distributed Bass kernel for 8 TRN2 NeuronCores (v2, bf16).

Data-parallel over batch: B=8 samples -> 1 per core. Weights replicated.

Per-core math (S=2048, D=1024, H=16, HD=64):
  q = X @ W_qv + b_qv                 ; v = q
  k = X @ W_k + b_k
  alphas = softmax_h((q @ Wq_s + bq_s) * sc)       sc = 1/sqrt(HD)
  gq[d]  = sum_s alphas[s, h(d)] * q[s, d]         h(d) = d // 64
  p = k * gq                                        (broadcast over s)
  betas  = softmax_h((p @ Wk_s + bk_s) * sc)
  gk[d]  = gq[d] * sum_s betas[s, h(d)] * k[s, d]
  out = q + (q*gk) @ W_r + b_r

Key algebraic folds (avoid transposed-layout round trips):
  - logits_b = k @ (diag(gq) Wk_s) + bk_s          (p never materialized)
  - out      = q @ (I + diag(gk) W_r) + b_r        (residual folded into W_r)
  - gq_raw   = W_qv^T (X^T alphas) + b_qv colsum(alphas)   (q_nat never needed)

Layout: activations transposed (qT[d, s]) so big matmuls use natural weights
[c, d] as stationary and XT[c, s] as moving operand.  XT comes from the DMA
xbar transpose (bf16).  All matmuls bf16 (FWL weight loads) with f32 PSUM.
Host pre-casts X and weights to bf16.
"""

import math
import os
from contextlib import ExitStack

import numpy as np

B, S, D, H = 8, 2048, 1024, 16
HD = D // H
SCALE = 1.0 / math.sqrt(HD)
NCORES = 8
P = 128
NDB = D // P      # 8 d-blocks
NSB = S // P      # 16 s-blocks
NCC = D // P      # 8 contraction chunks
SH = 512          # psum free width for big matmuls
NSH = S // SH     # 4
NDH = D // SH     # 2

_CACHE = {}


def _build():
    import concourse.bacc as bacc
    import concourse.tile as tile
    import concourse.mybir as mybir

    f32 = mybir.dt.float32
    bf16 = mybir.dt.bfloat16
    AF = mybir.ActivationFunctionType
    ALU = mybir.AluOpType

    nc = bacc.Bacc("TRN2", target_bir_lowering=False, debug=False,
                   num_devices=NCORES)

    # bf16 inputs (host pre-cast)
    X = nc.dram_tensor("Xb", [S, D], bf16, kind="ExternalInput").ap()
    XT = nc.dram_tensor("XTb", [D, S], bf16, kind="ExternalInput").ap()
    W_qv = nc.dram_tensor("W_qvb", [D, D], bf16, kind="ExternalInput").ap()
    W_k = nc.dram_tensor("W_kb", [D, D], bf16, kind="ExternalInput").ap()
    W_r = nc.dram_tensor("W_rb", [D, D], bf16, kind="ExternalInput").ap()
    Wq_s = nc.dram_tensor("Wq_sb", [D, H], bf16, kind="ExternalInput").ap()
    Wk_s = nc.dram_tensor("Wk_sb", [D, H], bf16, kind="ExternalInput").ap()
    bq_sb = nc.dram_tensor("bq_sbf", [H], bf16, kind="ExternalInput").ap()
    bk_sb = nc.dram_tensor("bk_sbf", [H], bf16, kind="ExternalInput").ap()
    br_b = nc.dram_tensor("b_rbf", [D], bf16, kind="ExternalInput").ap()
    bqv_b = nc.dram_tensor("b_qvbf", [D], bf16, kind="ExternalInput").ap()
    bkv_b = nc.dram_tensor("b_kbf", [D], bf16, kind="ExternalInput").ap()
    # f32 biases for per-partition epilogues
    b_qv = nc.dram_tensor("b_qv", [D], f32, kind="ExternalInput").ap()
    b_k = nc.dram_tensor("b_k", [D], f32, kind="ExternalInput").ap()
    OUT = nc.dram_tensor("out", [S, D], f32, kind="ExternalOutput").ap()

    with tile.TileContext(nc) as tc, ExitStack() as ctx:
        sbp = ctx.enter_context(tc.tile_pool(name="sbp", bufs=1))
        psp = ctx.enter_context(tc.tile_pool(name="psp", bufs=1, space="PSUM"))

        def st(shape, dt_, tag, bufs=1):
            return sbp.tile(shape, dt_, tag=tag, bufs=bufs, name=tag)

        def pt_(shape, tag, bufs):
            return psp.tile(shape, f32, tag=tag, bufs=bufs, name=tag)


        # ---------- small persistent intermediates ----------
        aE = st([P, NSB * H], f32, "aE")
        Za = st([P, NSB], f32, "Za")
        rZa = st([P, NSB], f32, "rZa")
        alpha = st([P, NSB * H], bf16, "alpha")
        bEx = st([P, NSB * H], f32, "bEx")
        Zb = st([P, NSB], f32, "Zb")
        rZb = st([P, NSB], f32, "rZb")
        beta = st([P, NSB * H], bf16, "beta")
        Asb_q = st([P, NCC * H], bf16, "Asbq")
        Ssb_q = st([1, H], bf16, "Ssbq")
        Asb_k = st([P, NCC * H], bf16, "Asbk")
        Ssb_k = st([1, H], bf16, "Ssbk")
        gq = st([P, NDB], f32, "gq")
        gkd = st([P, NDB], f32, "gkd")
        gk = st([P, NDB], f32, "gk")

        # ---------- big persistent activations / resident data ----------
        xt = st([P, NCC * S], bf16, "xt")   # X^T, chunk cc at cols cc*S
        qt = st([P, NDB * S], bf16, "qt")   # q^T, d-block j at cols j*S
        kt = st([P, NDB * S], bf16, "kt")   # k^T
        xnat = st([P, NSB * D], bf16, "xnat")  # natural X, s-block si at si*D
        wqv_all = st([P, NCC * D], bf16, "wqv_all")
        wk_all = st([P, NCC * D], bf16, "wk_all")
        wr_all = st([P, NCC * D], bf16, "wr_all")

        # Startup-critical loads are sliced to match the q-projection loop
        # order (sh-outer): the first psum tile needs only XT[:, sh=0] (1MB)
        # + W_qv[:, j=0] (0.25MB).  Slices alternate between the SP HWDGE
        # queue and the gpsimd SWDGE queue; xnat + W_k follow on SP, W_r +
        # small tensors on ACT.
        HB = NCC // 2

        def wqv_slice(j, eng):
            eng.dma_start(
                wqv_all[:].rearrange("p (cc d) -> p cc d", cc=NCC)
                [:, :, j * P:(j + 1) * P],
                W_qv[:, j * P:(j + 1) * P].rearrange("(cc p) m -> p cc m", p=P))

        def xt_slice(sh, half, eng):
            lo = half * HB
            eng.dma_start(
                xt[:, lo * S:(lo + HB) * S].rearrange("p (cc s) -> p cc s", cc=HB)
                [:, :, sh * SH:(sh + 1) * SH],
                XT[lo * P:(lo + HB) * P, sh * SH:(sh + 1) * SH]
                .rearrange("(cc p) s -> p cc s", p=P))

        for sh in range(NSH):
            wqv_slice(2 * sh, nc.sync)
            wqv_slice(2 * sh + 1, nc.gpsimd)
            xt_slice(sh, 0, nc.sync)
            xt_slice(sh, 1, nc.gpsimd)
        nc.sync.dma_start(
            xnat[:].rearrange("p (si c) -> p si c", si=NSB),
            X.rearrange("(si p) c -> p si c", p=P))
        nc.sync.dma_start(
            wk_all[:].rearrange("p (cc d) -> p cc d", cc=NCC),
            W_k.rearrange("(cc p) d -> p cc d", p=P))

        # ---------- constants (gpsimd, after its DMAs are queued) ----------
        ones_row = st([1, P], bf16, "ones_row")
        nc.gpsimd.memset(ones_row[:], 1.0)
        ones_col = st([P, 1], bf16, "ones_col")
        nc.gpsimd.memset(ones_col[:], 1.0)
        ones16 = st([16, 1], bf16, "ones16")
        nc.gpsimd.memset(ones16[:], 1.0)
        eye_bf = st([P, P], bf16, "eye_bf")
        nc.gpsimd.memset(eye_bf[:], 1.0)
        nc.gpsimd.affine_select(eye_bf[:], eye_bf[:], pattern=[[1, P]],
                                compare_op=ALU.is_equal, fill=0.0,
                                base=0, channel_multiplier=-1)
        # head-selector mask: hmask[h, d] = 1 iff h == d//64   (16 partitions)
        # built as [d >= 64h] - [d >= 64(h+1)]  (walrus implements is_ge only)
        hmask = st([16, D], f32, "hmask")
        hm2 = st([16, D], f32, "hm2")
        nc.gpsimd.memset(hmask[:], 1.0)
        nc.gpsimd.memset(hm2[:], 1.0)
        nc.gpsimd.affine_select(hmask[:], hmask[:], pattern=[[1, D]],
                                compare_op=ALU.is_ge, fill=0.0,
                                base=0, channel_multiplier=-HD)
        nc.gpsimd.affine_select(hm2[:], hm2[:], pattern=[[1, D]],
                                compare_op=ALU.is_ge, fill=0.0,
                                base=-HD, channel_multiplier=-HD)
        nc.vector.tensor_sub(hmask[:], hmask[:], hm2[:])

        bqv_pp = st([P, NDB], f32, "bqv_pp")
        nc.scalar.dma_start(bqv_pp[:], b_qv.rearrange("(j p) -> p j", p=P))
        bk_pp = st([P, NDB], f32, "bk_pp")
        nc.scalar.dma_start(bk_pp[:], b_k.rearrange("(j p) -> p j", p=P))
        bqv_row = st([1, D], bf16, "bqv_row")
        nc.scalar.dma_start(bqv_row[:], bqv_b.unsqueeze(0))
        bk_row = st([1, D], bf16, "bk_row")
        nc.scalar.dma_start(bk_row[:], bkv_b.unsqueeze(0))
        br_row = st([1, D], bf16, "br_row")
        nc.scalar.dma_start(br_row[:], br_b.unsqueeze(0))
        bqs_row = st([1, H], bf16, "bqs_row")
        nc.scalar.dma_start(bqs_row[:], bq_sb.unsqueeze(0))
        bks_row = st([1, H], bf16, "bks_row")
        nc.scalar.dma_start(bks_row[:], bk_sb.unsqueeze(0))
        wqs_sb = st([P, NDB * H], bf16, "wqs_sb")
        nc.scalar.dma_start(wqs_sb[:].rearrange("p (j h) -> p j h", j=NDB),
                          Wq_s.rearrange("(j p) h -> p j h", p=P))
        wks_sb = st([P, NDB * H], bf16, "wks_sb")   # becomes diag(gq)-scaled
        nc.scalar.dma_start(wks_sb[:].rearrange("p (j h) -> p j h", j=NDB),
                          Wk_s.rearrange("(j p) h -> p j h", p=P))

        nc.scalar.dma_start(
            wr_all[:].rearrange("p (cc d) -> p cc d", cc=NCC),
            W_r.rearrange("(cc p) d -> p cc d", p=P))

        # ---------- phases 2+3: qT / kT projections ----------
        def project(wall, bias_pp, dst, sh_outer=False):
            loop = ([(j, sh) for sh in range(NSH) for j in range(NDB)]
                    if sh_outer else
                    [(j, sh) for j in range(NDB) for sh in range(NSH)])
            for j, sh in loop:
                ps = pt_([P, SH], "big", 3)
                for cc in range(NCC):
                    nc.tensor.matmul(
                        ps[:], wall[:, cc * D + j * P: cc * D + j * P + P],
                        xt[:, cc * S + sh * SH: cc * S + sh * SH + SH],
                        start=(cc == 0), stop=(cc == NCC - 1))
                nc.vector.tensor_scalar(
                    dst[:, j * S + sh * SH: j * S + sh * SH + SH], ps[:],
                    bias_pp[:, j:j + 1], None, ALU.add)

        project(wqv_all, bqv_pp, qt, sh_outer=True)

        # ---------- logits + softmax (shared for alphas / betas) ----------
        def softmax_weights(src_t, w16, brow, eE, Z, rZ, wout, pe_filler=None):
            lg = pt_([P, NSB * H], "small", 3)
            for sb in range(NSB):
                for j in range(NDB):
                    nc.tensor.matmul(
                        lg[:, sb * H:(sb + 1) * H],
                        src_t[:, j * S + sb * P: j * S + sb * P + P],
                        w16[:, j * H:(j + 1) * H],
                        start=(sb == 0 and j == 0), stop=False)
                nc.tensor.matmul(
                    lg[:, sb * H:(sb + 1) * H],
                    ones_row[:1, :], brow[:1, :],
                    start=False, stop=(sb == NSB - 1))
            if pe_filler is not None:
                pe_filler()
            nc.scalar.activation(eE[:], lg[:], AF.Exp, bias=0.0, scale=SCALE)
            nc.vector.reduce_sum(
                Z[:].unsqueeze(2),
                eE[:].rearrange("p (sb h) -> p sb h", sb=NSB),
                axis=mybir.AxisListType.X)
            nc.vector.reciprocal(rZ[:], Z[:])
            nc.vector.tensor_tensor(
                wout[:].rearrange("p (sb h) -> p sb h", sb=NSB),
                eE[:].rearrange("p (sb h) -> p sb h", sb=NSB),
                rZ[:].unsqueeze(2).broadcast_to([P, NSB, H]),
                ALU.mult)

        # ---------- gq_raw = W^T (X^T w) + b colsum(w), extract diagonal ----------
        def weighted_sum(weights_sb, wall, b_row, Asb, Ssb, g):
            Aps = pt_([P, NCC * H], "small", 3)
            Sps = pt_([1, H], "small", 3)
            for si in range(NSB):
                for cb in range(NCC):
                    nc.tensor.matmul(
                        Aps[:, cb * H:(cb + 1) * H],
                        xnat[:, si * D + cb * P: si * D + cb * P + P],
                        weights_sb[:, si * H:(si + 1) * H],
                        start=(si == 0 and cb == 0),
                        stop=(si == NSB - 1 and cb == NCC - 1))
                nc.tensor.matmul(
                    Sps[:1, :], ones_col[:, :1],
                    weights_sb[:, si * H:(si + 1) * H],
                    start=(si == 0), stop=(si == NSB - 1))
            nc.vector.tensor_copy(Asb[:], Aps[:])
            nc.vector.tensor_copy(Ssb[:1, :], Sps[:1, :])
            # gq_rawT[h, d] = (A^T W)[h, d] + S[h] b[d]   (16-col stationary)
            grawT = pt_([16, D], "grawT", 1)
            for dh in range(NDH):
                for cc in range(NCC):
                    nc.tensor.matmul(
                        grawT[:, dh * SH:(dh + 1) * SH],
                        Asb[:, cc * H:(cc + 1) * H],
                        wall[:, cc * D + dh * SH: cc * D + dh * SH + SH],
                        start=(cc == 0), stop=False)
                nc.tensor.matmul(
                    grawT[:, dh * SH:(dh + 1) * SH],
                    Ssb[:1, :], b_row[:1, dh * SH:(dh + 1) * SH],
                    start=False, stop=True)
            # diagonal extract: g[d] = gq_rawT[d//64, d]  via mask + ones-matmul
            msk = st([16, D], bf16, "msk", bufs=2)
            nc.vector.tensor_tensor(msk[:], grawT[:], hmask[:], ALU.mult)
            gps = pt_([P, NDB], "small", 3)
            for j in range(NDB):
                nc.tensor.matmul(
                    gps[:, j:j + 1],
                    msk[:, j * P:(j + 1) * P], ones16[:, :1],
                    start=(j == 0), stop=(j == NDB - 1))
            nc.vector.tensor_copy(g[:], gps[:])

        # alphas (k-projection emitted between logits and exp so the PE has
        # dense work while ACT/DVE run the softmax tail)
        softmax_weights(qt, wqs_sb, bqs_row, aE, Za, rZa, alpha,
                        pe_filler=lambda: project(wk_all, bk_pp, kt))
        # gq
        weighted_sum(alpha, wqv_all, bqv_row, Asb_q, Ssb_q, gq)
        # fold gq into Wk_s  ->  logits_b from kT directly
        for j in range(NDB):
            nc.vector.tensor_scalar(
                wks_sb[:, j * H:(j + 1) * H], wks_sb[:, j * H:(j + 1) * H],
                gq[:, j:j + 1], None, ALU.mult)
        # betas  (logits_b = k @ (diag(gq) Wk_s) + bk_s)
        softmax_weights(kt, wks_sb, bks_row, bEx, Zb, rZb, beta)
        # gk = gq * (W_k^T (X^T beta) + b_k colsum(beta)) diag
        weighted_sum(beta, wk_all, bk_row, Asb_k, Ssb_k, gkd)
        nc.vector.tensor_mul(gk[:], gq[:], gkd[:])

        # ---------- fold residual + gk into W_r:  W_r' = diag(gk) W_r + I ----------
        for cc in range(NCC):
            nc.vector.tensor_scalar(
                wr_all[:, cc * D:(cc + 1) * D], wr_all[:, cc * D:(cc + 1) * D],
                gk[:, cc:cc + 1], None, ALU.mult)
            nc.vector.tensor_add(
                wr_all[:, cc * D + cc * P: cc * D + (cc + 1) * P],
                wr_all[:, cc * D + cc * P: cc * D + (cc + 1) * P], eye_bf[:])

        # ---------- final: out = q @ W_r' + b_r ----------
        for sb in range(NSB):
            for dh in range(NDH):
                ps = pt_([P, SH], "big", 3)
                nc.tensor.matmul(
                    ps[:], ones_row[:1, :],
                    br_row[:1, dh * SH:(dh + 1) * SH],
                    start=True, stop=False)
                for cc in range(NCC):
                    nc.tensor.matmul(
                        ps[:], qt[:, cc * S + sb * P: cc * S + sb * P + P],
                        wr_all[:, cc * D + dh * SH: cc * D + dh * SH + SH],
                        start=False, stop=(cc == NCC - 1))
                ob = st([P, SH], f32, "ob", bufs=3)
                nc.scalar.copy(ob[:], ps[:])
                nc.sync.dma_start(
                    OUT[sb * P:(sb + 1) * P, dh * SH:(dh + 1) * SH], ob[:])

    nc.compile()
    return nc


def _get_nc():
    if "nc" not in _CACHE:
        _CACHE["nc"] = _build()
    return _CACHE["nc"]


def _prep_inputs(inputs):
    import ml_dtypes
    bf = ml_dtypes.bfloat16

    def f(k):
        return np.ascontiguousarray(np.asarray(inputs[k], dtype=np.float32))

    def c(a):
        return np.ascontiguousarray(np.asarray(a, dtype=np.float32).astype(bf))

    common = {
        "W_qvb": c(inputs["W_qv"]), "W_kb": c(inputs["W_k"]),
        "W_rb": c(inputs["W_r"]), "Wq_sb": c(inputs["Wq_s"]),
        "Wk_sb": c(inputs["Wk_s"]), "bq_sbf": c(inputs["bq_s"]),
        "bk_sbf": c(inputs["bk_s"]), "b_rbf": c(inputs["b_r"]),
        "b_qvbf": c(inputs["b_qv"]), "b_kbf": c(inputs["b_k"]),
        "b_qv": f("b_qv"), "b_k": f("b_k"),
    }
    in_maps = []
    for b in range(NCORES):
        m = dict(common)
        xb = np.asarray(inputs["X"][b], dtype=np.float32)
        m["Xb"] = c(xb)
        m["XTb"] = c(xb.T)
        in_maps.append(m)
    return in_maps


def run(inputs, trace=False):
    from concourse.bass_utils import run_bass_kernel_spmd

    nc = _get_nc()
    in_maps = _prep_inputs(inputs)
    res = run_bass_kernel_spmd(nc, in_maps, core_ids=list(range(NCORES)),
                               trace=trace)
    _CACHE["last_results"] = res
    out = np.stack([res.results[b]["out"] for b in range(NCORES)], axis=0)
    return out


def kernel(**inputs):
    trace = os.environ.get("KTRACE", "0") == "1"
    return run(inputs, trace=trace)
